# revision 2
# baseline (speedup 1.0000x reference)
"""Routed quantized MoE eval kernel for 8 Trainium2 NeuronCores.

Hybrid pair-parallel strategy (v2, replaces 8-way expert-parallel):
- Cores are grouped in 4 pairs; pair m owns tokens [512m, 512m+512).
  Within a pair, the even core computes experts {0..3} and shared-FFN
  rows [0:1024), the odd core experts {4..7} and rows [1024:2048).
  The pair-sum is taken by a 2-rank ReduceScatter per 512-column half
  of the output (first RS hidden under the second half's compute).
- Expert order is PERMUTED per parity (odd cores see experts
  [4,5,6,7,0,1,2,3] in router weights, alpha and weight uploads), so
  the SPMD program always works on combine columns 0..3.  Top-2
  selection and softmax are permutation-invariant.
- All matmuls keep 512-wide moving operands (PE instruction overhead
  ~50ns each - fewer, wider matmuls win).  Router uses the same
  split-precision fp16 trick as before (3 exact fp16 products in one
  fp32 PSUM).
- Per-(token-tile, dd) combine: contrib = sum_e ca_e*alpha_e*pse_e
  + (1-s)*pss via a chain of scalar_tensor_tensor ops on the DVE.
- Weights are host-dequantized (scale-folded), transposed, and packed
  to a [128, k*C] tile layout so every tensor is one large DMA.
"""

import numpy as np
from contextlib import ExitStack

import concourse.bass as bass
import concourse.tile as tile
from concourse import bacc, mybir
from concourse.bass_utils import run_bass_kernel_spmd

NCORES = 8
B, S, D = 2, 1024, 1024
T = B * S                      # 2048 tokens
DF_E, DF_S, E = 512, 2048, 8
NPAIR = 4
CT = T // NPAIR                # 512 tokens per pair
TT = CT // 128                 # 4 token tiles
NE = 4                         # experts per core
FS = DF_S // 2                 # 1024 shared-ffn rows per core
KD = D // 128                  # 8 k-tiles over hidden dim
KF = DF_E // 128               # 4 k-tiles over expert ffn dim
KS = FS // 128                 # 8 k-tiles over shared ffn shard
FE = DF_E // 128               # 4 f-tiles per expert

F16 = mybir.dt.float16
F32 = mybir.dt.float32
ACTF = mybir.ActivationFunctionType
ALU = mybir.AluOpType

PAIR_GROUPS = [[2 * m, 2 * m + 1] for m in range(NPAIR)]

_CACHE = {}


def _build():
    nc = bacc.Bacc(
        "TRN2", target_bir_lowering=False, debug=False, num_devices=NCORES
    )

    # all weight tensors are packed [128, k*C]: column block k holds rows
    # [128k, 128k+128) of the logical [k*128, C] matrix
    xP = nc.dram_tensor("xP", [128, KD * CT], F16, kind="ExternalInput").ap()
    xRP = nc.dram_tensor("xRP", [128, KD * CT], F16, kind="ExternalInput").ap()
    rwP = nc.dram_tensor("rwP", [128, KD * 2 * E], F16, kind="ExternalInput").ap()
    GQ = nc.dram_tensor("GQ", [128, NE * KD * DF_E], F16, kind="ExternalInput").ap()
    UQ = nc.dram_tensor("UQ", [128, NE * KD * DF_E], F16, kind="ExternalInput").ap()
    DQ = nc.dram_tensor("DQ", [128, NE * KF * D], F16, kind="ExternalInput").ap()
    WG = nc.dram_tensor("WG", [128, KD * FS], F16, kind="ExternalInput").ap()
    WU = nc.dram_tensor("WU", [128, KD * FS], F16, kind="ExternalInput").ap()
    WD = nc.dram_tensor("WD", [128, KS * D], F16, kind="ExternalInput").ap()
    AUX = nc.dram_tensor("AUX", [128, E], F32, kind="ExternalInput").ap()
    OUT = nc.dram_tensor("OUT", [CT // 2, D], F16, kind="ExternalOutput").ap()

    with ExitStack() as ctx:
        tc = ctx.enter_context(tile.TileContext(nc))
        wres = ctx.enter_context(tc.tile_pool(name="wres", bufs=1))
        hp = ctx.enter_context(tc.tile_pool(name="hp", bufs=1))
        work = ctx.enter_context(tc.tile_pool(name="work", bufs=2))
        rt = ctx.enter_context(tc.tile_pool(name="rt", bufs=1))
        ps_gu = ctx.enter_context(tc.tile_pool(name="ps_gu", bufs=3, space="PSUM"))
        ps_dn = ctx.enter_context(tc.tile_pool(name="ps_dn", bufs=3, space="PSUM"))
        ps_r = ctx.enter_context(tc.tile_pool(name="ps_r", bufs=1, space="PSUM"))
        dram = ctx.enter_context(tc.tile_pool(name="dram", bufs=1, space="DRAM"))

        from concourse.masks import make_identity

        # warmup ReduceScatter FIRST on the in-order gpsimd queue: the
        # ncfw cold-start begins at trigger time, so trigger ASAP
        wu_in = dram.tile([2, 128], F16, tag="wuin")
        wu_out = dram.tile([1, 128], F16, tag="wuout")
        nc.sync.dma_start(wu_in[:], xP[0:2, 0:128])
        nc.gpsimd.collective_compute(
            "ReduceScatter",
            ALU.add,
            replica_groups=PAIR_GROUPS,
            ins=[wu_in.opt()],
            outs=[wu_out.opt()],
        )
        ident = wres.tile([128, 128], F32, tag="ident")
        make_identity(nc, ident[:])
        # dummy matmuls during the initial DMA wait: warms the PE clock.
        # Reuses a ps_gu-pool generation to stay within the 8 PSUM banks.
        wu_ps = ps_gu.tile([128, CT], F32, tag="psgu")
        for _ in range(24):
            nc.tensor.matmul(
                wu_ps[:, 0:128], ident[:], ident[:], start=True, stop=True
            )

        # ---- input DMAs, spread across 3 queues ----------------------
        # (Measured best layout: the sync queue carries the router-
        # critical x + gate weights, scalar the residual + up weights,
        # gpsimd the shared + down weights.  A single ordered queue was
        # tried and measured slower end-to-end.)
        rw = wres.tile([128, KD * 2 * E], F16, tag="rw")
        nc.sync.dma_start(rw[:], rwP[:])
        aux_sb = wres.tile([128, E], F32, tag="aux")
        nc.sync.dma_start(aux_sb[:], AUX[:])
        alpha_bc = aux_sb[:, 0:E]

        xt = wres.tile([128, KD * CT], F16, tag="xt")
        nc.sync.dma_start(xt[:], xP[:])
        xr = wres.tile([128, KD * CT], F16, tag="xr")
        nc.scalar.dma_start(xr[:], xRP[:])

        gq = wres.tile([128, NE * KD * DF_E], F16, tag="gq")
        uq = wres.tile([128, NE * KD * DF_E], F16, tag="uq")
        dq = wres.tile([128, NE * KF * D], F16, tag="dq")
        wg = wres.tile([128, KD * FS], F16, tag="wg")
        wu_t = wres.tile([128, KD * FS], F16, tag="wu")
        wd = wres.tile([128, KS * D], F16, tag="wd")
        EW = KD * DF_E  # 4096 cols per expert in gq/uq
        DW = KF * D     # 4096 cols per expert in dq
        # e0/e1 gate+up early (sync+scalar), rest on gpsimd
        nc.sync.dma_start(gq[:, 0:EW], GQ[:, 0:EW])
        nc.scalar.dma_start(uq[:, 0:EW], UQ[:, 0:EW])
        nc.sync.dma_start(gq[:, EW : 2 * EW], GQ[:, EW : 2 * EW])
        nc.scalar.dma_start(uq[:, EW : 2 * EW], UQ[:, EW : 2 * EW])
        nc.sync.dma_start(gq[:, 2 * EW : 3 * EW], GQ[:, 2 * EW : 3 * EW])
        nc.scalar.dma_start(uq[:, 2 * EW : 3 * EW], UQ[:, 2 * EW : 3 * EW])
        nc.sync.dma_start(gq[:, 3 * EW : 4 * EW], GQ[:, 3 * EW : 4 * EW])
        nc.scalar.dma_start(uq[:, 3 * EW : 4 * EW], UQ[:, 3 * EW : 4 * EW])
        nc.gpsimd.dma_start(wg[:], WG[:])
        nc.gpsimd.dma_start(wu_t[:], WU[:])
        for e in range(NE):
            nc.gpsimd.dma_start(dq[:, e * DW : (e + 1) * DW], DQ[:, e * DW : (e + 1) * DW])
        nc.gpsimd.dma_start(wd[:], WD[:])

        def xk(k):
            return xt[:, k * CT : (k + 1) * CT]

        def xrk(k):
            return xr[:, k * CT : (k + 1) * CT]

        # ---- router: split-precision fp16 logits ---------------------
        ps_lt = ps_r.tile([E, CT], F32, tag="psr_lt")
        for k in range(KD):
            nc.tensor.matmul(
                ps_lt[:], rw[:, k * 2 * E : k * 2 * E + E], xk(k),
                start=(k == 0), stop=False,
            )
        for k in range(KD):
            nc.tensor.matmul(
                ps_lt[:], rw[:, k * 2 * E + E : (k + 1) * 2 * E], xk(k),
                start=False, stop=False,
            )
        for k in range(KD):
            nc.tensor.matmul(
                ps_lt[:], rw[:, k * 2 * E : k * 2 * E + E], xrk(k),
                start=False, stop=(k == KD - 1),
            )

        # ---- combine weights (top-2 softmax * alpha) -----------------
        Lt = rt.tile([E, CT], F32, tag="Lt")
        nc.vector.tensor_copy(Lt[:], ps_lt[:])
        ps_l = ps_r.tile([128, TT * E], F32, tag="psr_l")
        for j in range(TT):
            nc.tensor.transpose(
                ps_l[:, j * E : (j + 1) * E],
                Lt[:, j * 128 : (j + 1) * 128],
                ident[0:E, 0:E],
            )
        L = rt.tile([128, TT * E], F32, tag="L")
        nc.vector.tensor_copy(L[:], ps_l[:])
        L3 = L[:].rearrange("p (j e) -> p j e", e=E)

        def bc(t):  # [128, TT] -> [128, TT, E] free-axis broadcast
            return t[:, :, None].broadcast_to([128, TT, E])

        m1 = rt.tile([128, TT], F32, tag="m1")
        nc.vector.tensor_reduce(m1[:], L3, mybir.AxisListType.X, ALU.max)
        mask1 = rt.tile([128, TT * E], F32, tag="mask1")
        mask1_3 = mask1[:].rearrange("p (j e) -> p j e", e=E)
        nc.vector.tensor_tensor(mask1_3, L3, bc(m1), op=ALU.is_ge)
        L2 = rt.tile([128, TT * E], F32, tag="L2")
        nc.vector.scalar_tensor_tensor(
            L2[:], mask1[:], -1e30, L[:], ALU.mult, ALU.add
        )
        L2_3 = L2[:].rearrange("p (j e) -> p j e", e=E)
        m2 = rt.tile([128, TT], F32, tag="m2")
        nc.vector.tensor_reduce(m2[:], L2_3, mybir.AxisListType.X, ALU.max)
        mask2 = rt.tile([128, TT * E], F32, tag="mask2")
        mask2_3 = mask2[:].rearrange("p (j e) -> p j e", e=E)
        nc.vector.tensor_tensor(mask2_3, L2_3, bc(m2), op=ALU.is_ge)
        # softmax over {m1, m2}: w1 = sigmoid(m1 - m2), w2 = 1 - w1
        dlt = rt.tile([128, TT], F32, tag="dlt")
        nc.vector.tensor_sub(dlt[:], m1[:], m2[:])
        w1 = rt.tile([128, TT], F32, tag="w1")
        nc.scalar.activation(w1[:], dlt[:], ACTF.Sigmoid)
        w2 = rt.tile([128, TT], F32, tag="w2")
        nc.vector.tensor_scalar(w2[:], w1[:], -1.0, 1.0, ALU.mult, ALU.add)
        caw = rt.tile([128, TT * E], F32, tag="caw")
        caw3 = caw[:].rearrange("p (j e) -> p j e", e=E)
        nc.vector.tensor_tensor(caw3, mask2_3, bc(w2), op=ALU.mult)
        t1 = rt.tile([128, TT * E], F32, tag="t1")
        t1_3 = t1[:].rearrange("p (j e) -> p j e", e=E)
        nc.vector.tensor_tensor(t1_3, mask1_3, bc(w1), op=ALU.mult)
        nc.vector.tensor_add(caw[:], caw[:], t1[:])
        # scale by alpha (broadcast over token-tiles) and reduce
        ca_a = rt.tile([128, TT * E], F32, tag="ca_a")
        ca_a3 = ca_a[:].rearrange("p (j e) -> p j e", e=E)
        alpha3 = alpha_bc[:, None, :].broadcast_to([128, TT, E])
        nc.vector.tensor_tensor(ca_a3, caw3, alpha3, op=ALU.mult)
        s = rt.tile([128, TT], F32, tag="s")
        nc.vector.tensor_reduce(s[:], ca_a3, mybir.AxisListType.X, ALU.add)
        om_all = rt.tile([128, TT], F32, tag="om")
        nc.vector.tensor_scalar(om_all[:], s[:], -1.0, 1.0, ALU.mult, ALU.add)

        # ---- gate/up + SwiGLU for 4 experts + shared half ------------
        def gu_block(wsrc_g, wsrc_u, base, f, tag):
            """one 128-row f-tile of gate/up + silu -> h tile (f16)."""
            psg = ps_gu.tile([128, CT], F32, tag="psgu")
            for k in range(KD):
                nc.tensor.matmul(
                    psg[:],
                    wsrc_g[:, base + k * stride + f * 128 : base + k * stride + f * 128 + 128],
                    xk(k),
                    start=(k == 0),
                    stop=(k == KD - 1),
                )
            psu = ps_gu.tile([128, CT], F32, tag="psgu")
            for k in range(KD):
                nc.tensor.matmul(
                    psu[:],
                    wsrc_u[:, base + k * stride + f * 128 : base + k * stride + f * 128 + 128],
                    xk(k),
                    start=(k == 0),
                    stop=(k == KD - 1),
                )
            sig = work.tile([128, CT], F32, tag="sig")
            nc.scalar.activation(sig[:], psg[:], ACTF.Sigmoid)
            sil = work.tile([128, CT], F32, tag="sil")
            nc.vector.tensor_mul(sil[:], sig[:], psg[:])
            h = hp.tile([128, CT], F16, tag=tag)
            nc.vector.tensor_mul(h[:], sil[:], psu[:])
            return h

        he = {}
        stride = DF_E  # within one expert block, k-tiles are DF_E wide
        for e in range(NE):
            for f in range(FE):
                he[(e, f)] = gu_block(gq, uq, e * EW, f, f"he{e}_{f}")
        hs = {}
        stride = FS
        for f in range(KS):
            hs[f] = gu_block(wg, wu_t, 0, f, f"hs{f}")

        # ---- down projections + combine, 2 column halves -------------
        ND = D // 512
        rs_in = [dram.tile([CT, 512], F16, tag=f"rsin{dd}", name=f"rsin{dd}") for dd in range(ND)]
        rs_out = [dram.tile([CT // 2, 512], F16, tag=f"rsout{dd}", name=f"rsout{dd}") for dd in range(ND)]
        for dd in range(ND):
            for j in range(TT):
                pse = []
                for e in range(NE):
                    p = ps_dn.tile([128, 512], F32, tag="psd")
                    for k in range(KF):
                        nc.tensor.matmul(
                            p[:],
                            he[(e, k)][:, j * 128 : (j + 1) * 128],
                            dq[:, e * DW + k * D + dd * 512 : e * DW + k * D + dd * 512 + 512],
                            start=(k == 0),
                            stop=(k == KF - 1),
                        )
                    pse.append(p)
                pss = ps_dn.tile([128, 512], F32, tag="psd")
                for k in range(KS):
                    nc.tensor.matmul(
                        pss[:],
                        hs[k][:, j * 128 : (j + 1) * 128],
                        wd[:, k * D + dd * 512 : k * D + dd * 512 + 512],
                        start=(k == 0),
                        stop=(k == KS - 1),
                    )
                # contrib = sum_e ca_e*pse_e + (1-s)*pss  (ca cols 0..3)
                def cacol(e):
                    return ca_a[:, j * E + e : j * E + e + 1]

                acc_a = work.tile([128, 512], F32, tag="acc_a")
                nc.vector.tensor_scalar(acc_a[:], pse[0][:], cacol(0), None, ALU.mult)
                acc_b = work.tile([128, 512], F32, tag="acc_b")
                nc.vector.scalar_tensor_tensor(
                    acc_b[:], pse[1][:], cacol(1), acc_a[:], ALU.mult, ALU.add
                )
                acc_c = work.tile([128, 512], F32, tag="acc_a")
                nc.vector.scalar_tensor_tensor(
                    acc_c[:], pse[2][:], cacol(2), acc_b[:], ALU.mult, ALU.add
                )
                acc_d = work.tile([128, 512], F32, tag="acc_b")
                nc.vector.scalar_tensor_tensor(
                    acc_d[:], pse[3][:], cacol(3), acc_c[:], ALU.mult, ALU.add
                )
                contrib = work.tile([128, 512], F16, tag="contrib")
                nc.vector.scalar_tensor_tensor(
                    contrib[:],
                    pss[:],
                    om_all[:, j : j + 1],
                    acc_d[:],
                    ALU.mult,
                    ALU.add,
                )
                nc.scalar.dma_start(rs_in[dd][j * 128 : (j + 1) * 128, :], contrib[:])

            nc.gpsimd.collective_compute(
                "ReduceScatter",
                ALU.add,
                replica_groups=PAIR_GROUPS,
                ins=[rs_in[dd].opt()],
                outs=[rs_out[dd].opt()],
            )
            nc.gpsimd.dma_start(OUT[:, dd * 512 : (dd + 1) * 512], rs_out[dd][:])

    nc.compile()
    return nc


def _pack(a):
    """[k*128, C] row-major -> [128, k*C] tile-packed layout."""
    R, C = a.shape
    k = R // 128
    return np.ascontiguousarray(
        a.reshape(k, 128, C).transpose(1, 0, 2).reshape(128, k * C)
    )


def _prep_inputs(x, router_weight, sh_gate_w, sh_up_w, sh_down_w, gate_s,
                 up_s, down_s, alpha, gate_q, up_q, down_q):
    xf32 = np.asarray(x, dtype=np.float32).reshape(T, D).T  # [D, T]
    xf = xf32.astype(np.float16)
    xres = (xf32 - xf.astype(np.float32)).astype(np.float16)

    rw32 = np.asarray(router_weight, np.float32).T  # [D, E]

    # parity-dependent data: expert permutation + shared half
    par = {}
    for p in range(2):
        perm = list(range(E)) if p == 0 else list(range(4, 8)) + list(range(4))
        rw_p = rw32[:, perm]
        rw_hi = rw_p.astype(np.float16)
        rw_lo = (rw_p - rw_hi.astype(np.float32)).astype(np.float16)
        # interleave per k-tile: [D, 2E] with hi|lo per k after packing
        rwPk = np.concatenate([rw_hi, rw_lo], axis=1)  # [D, 16]
        gq_l, uq_l, dq_l = [], [], []
        for e in perm[:NE]:
            gw = (np.asarray(gate_q[e], np.float32)
                  * np.asarray(gate_s[e], np.float32)[:, None])  # [DF_E, D]
            uw = (np.asarray(up_q[e], np.float32)
                  * np.asarray(up_s[e], np.float32)[:, None])
            dw = (np.asarray(down_q[e], np.float32)
                  * np.asarray(down_s[e], np.float32)[:, None])  # [D, DF_E]
            gq_l.append(_pack(gw.T.astype(np.float16)))   # [128, KD*DF_E]
            uq_l.append(_pack(uw.T.astype(np.float16)))
            dq_l.append(_pack(dw.T.astype(np.float16)))   # [128, KF*D]
        aux = np.zeros((128, E), np.float32)
        aux[:, :] = np.asarray(alpha, np.float32)[perm][None, :]
        par[p] = {
            "rwP": _pack(rwPk.astype(np.float16)),
            "GQ": np.ascontiguousarray(np.concatenate(gq_l, axis=1)),
            "UQ": np.ascontiguousarray(np.concatenate(uq_l, axis=1)),
            "DQ": np.ascontiguousarray(np.concatenate(dq_l, axis=1)),
            "WG": _pack(
                np.asarray(sh_gate_w[p * FS : (p + 1) * FS], np.float32)
                .T.astype(np.float16)
            ),
            "WU": _pack(
                np.asarray(sh_up_w[p * FS : (p + 1) * FS], np.float32)
                .T.astype(np.float16)
            ),
            "WD": _pack(
                np.asarray(sh_down_w[:, p * FS : (p + 1) * FS], np.float32)
                .T.astype(np.float16)
            ),
            "AUX": aux,
        }
    xs = {}
    for m in range(NPAIR):
        xs[m] = {
            "xP": _pack(xf[:, m * CT : (m + 1) * CT]),
            "xRP": _pack(xres[:, m * CT : (m + 1) * CT]),
        }
    in_maps = []
    for c in range(NCORES):
        d = dict(par[c % 2])
        d.update(xs[c // 2])
        in_maps.append(d)
    return in_maps


def assemble(outs):
    """Per-core OUT [256, D]: core 2m+p holds tokens [512m+256p, +256)."""
    out = np.empty((T, D), np.float32)
    half = CT // 2
    for c in range(NCORES):
        m, p = c // 2, c % 2
        base = m * CT + p * half
        out[base : base + half] = np.asarray(outs[c])
    return out.reshape(B, S, D)


def kernel(x, router_weight, sh_gate_w, sh_up_w, sh_down_w, gate_s, up_s,
           down_s, alpha, gate_q, up_q, down_q, top_k, **run_kwargs):
    assert int(top_k) == 2, "kernel compiled for top_k=2"
    assert tuple(np.shape(x)) == (B, S, D)

    if "nc" not in _CACHE:
        _CACHE["nc"] = _build()
    nc = _CACHE["nc"]

    in_maps = _prep_inputs(
        x, router_weight, sh_gate_w, sh_up_w, sh_down_w, gate_s, up_s,
        down_s, alpha, gate_q, up_q, down_q,
    )
    res = run_bass_kernel_spmd(
        nc, in_maps, core_ids=list(range(NCORES)), **run_kwargs
    )
    _CACHE["last_results"] = res

    outs = [res.results[r]["OUT"] for r in range(NCORES)]
    return assemble(outs).astype(np.asarray(x).dtype)


# revision 3
# speedup vs baseline: 1.1007x; 1.1007x over previous
"""Routed quantized MoE eval kernel for 8 Trainium2 NeuronCores.

Hybrid pair-parallel strategy (v2, replaces 8-way expert-parallel):
- Cores are grouped in 4 pairs; pair m owns tokens [512m, 512m+512).
  Within a pair, the even core computes experts {0..3} and shared-FFN
  rows [0:1024), the odd core experts {4..7} and rows [1024:2048).
  The pair-sum is taken by a 2-rank ReduceScatter per 512-column half
  of the output (first RS hidden under the second half's compute).
- Expert order is PERMUTED per parity (odd cores see experts
  [4,5,6,7,0,1,2,3] in router weights, alpha and weight uploads), so
  the SPMD program always works on combine columns 0..3.  Top-2
  selection and softmax are permutation-invariant.
- All matmuls keep 512-wide moving operands (PE instruction overhead
  ~50ns each - fewer, wider matmuls win).  Router uses the same
  split-precision fp16 trick as before (3 exact fp16 products in one
  fp32 PSUM).
- Per-(token-tile, dd) combine: contrib = sum_e ca_e*alpha_e*pse_e
  + (1-s)*pss via a chain of scalar_tensor_tensor ops on the DVE.
- Weights are host-dequantized (scale-folded), transposed, and packed
  to a [128, k*C] tile layout so every tensor is one large DMA.
"""

import numpy as np
from contextlib import ExitStack

import concourse.bass as bass
import concourse.tile as tile
from concourse import bacc, mybir
from concourse.bass_utils import run_bass_kernel_spmd

NCORES = 8
B, S, D = 2, 1024, 1024
T = B * S                      # 2048 tokens
DF_E, DF_S, E = 512, 2048, 8
NPAIR = 4
CT = T // NPAIR                # 512 tokens per pair
TT = CT // 128                 # 4 token tiles
NE = 4                         # experts per core
FS = DF_S // 2                 # 1024 shared-ffn rows per core
KD = D // 128                  # 8 k-tiles over hidden dim
KF = DF_E // 128               # 4 k-tiles over expert ffn dim
KS = FS // 128                 # 8 k-tiles over shared ffn shard
FE = DF_E // 128               # 4 f-tiles per expert

F16 = mybir.dt.float16
F32 = mybir.dt.float32
ACTF = mybir.ActivationFunctionType
ALU = mybir.AluOpType

PAIR_GROUPS = [[2 * m, 2 * m + 1] for m in range(NPAIR)]

_CACHE = {}


def _build():
    nc = bacc.Bacc(
        "TRN2", target_bir_lowering=False, debug=False, num_devices=NCORES
    )

    # all weight tensors are packed [128, k*C]: column block k holds rows
    # [128k, 128k+128) of the logical [k*128, C] matrix
    xP = nc.dram_tensor("xP", [128, KD * CT], F16, kind="ExternalInput").ap()
    xRP = nc.dram_tensor("xRP", [128, KD * CT], F16, kind="ExternalInput").ap()
    rwP = nc.dram_tensor("rwP", [128, KD * 2 * E], F16, kind="ExternalInput").ap()
    GQ = nc.dram_tensor("GQ", [128, NE * KD * DF_E], F16, kind="ExternalInput").ap()
    UQ = nc.dram_tensor("UQ", [128, NE * KD * DF_E], F16, kind="ExternalInput").ap()
    DQ = nc.dram_tensor("DQ", [128, NE * KF * D], F16, kind="ExternalInput").ap()
    WG = nc.dram_tensor("WG", [128, KD * FS], F16, kind="ExternalInput").ap()
    WU = nc.dram_tensor("WU", [128, KD * FS], F16, kind="ExternalInput").ap()
    WD = nc.dram_tensor("WD", [128, KS * D], F16, kind="ExternalInput").ap()
    AUX = nc.dram_tensor("AUX", [128, E], F32, kind="ExternalInput").ap()
    OUT = nc.dram_tensor("OUT", [CT // 2, D], F16, kind="ExternalOutput").ap()

    with ExitStack() as ctx:
        tc = ctx.enter_context(tile.TileContext(nc))
        wres = ctx.enter_context(tc.tile_pool(name="wres", bufs=1))
        hp = ctx.enter_context(tc.tile_pool(name="hp", bufs=1))
        work = ctx.enter_context(tc.tile_pool(name="work", bufs=2))
        rt = ctx.enter_context(tc.tile_pool(name="rt", bufs=1))
        ps_gu = ctx.enter_context(tc.tile_pool(name="ps_gu", bufs=3, space="PSUM"))
        ps_dn = ctx.enter_context(tc.tile_pool(name="ps_dn", bufs=3, space="PSUM"))
        ps_r = ctx.enter_context(tc.tile_pool(name="ps_r", bufs=1, space="PSUM"))
        dram = ctx.enter_context(tc.tile_pool(name="dram", bufs=1, space="DRAM"))

        from concourse.masks import make_identity

        # warmup ReduceScatter FIRST on the in-order gpsimd queue: the
        # ncfw cold-start begins at trigger time, so trigger ASAP
        wu_in = dram.tile([2, 128], F16, tag="wuin")
        wu_out = dram.tile([1, 128], F16, tag="wuout")
        nc.sync.dma_start(wu_in[:], xP[0:2, 0:128])
        nc.gpsimd.collective_compute(
            "ReduceScatter",
            ALU.add,
            replica_groups=PAIR_GROUPS,
            ins=[wu_in.opt()],
            outs=[wu_out.opt()],
        )
        ident = wres.tile([128, 128], F32, tag="ident")
        make_identity(nc, ident[:])
        # dummy matmuls during the initial DMA wait: warms the PE clock.
        # Reuses a ps_gu-pool generation to stay within the 8 PSUM banks.
        wu_ps = ps_gu.tile([128, CT], F32, tag="psgu")
        for _ in range(24):
            nc.tensor.matmul(
                wu_ps[:, 0:128], ident[:], ident[:], start=True, stop=True
            )

        # ---- input DMAs: two queues, router-critical first ------------
        # DMA transfers serialize globally in arrival order, so the
        # sync/scalar queue heads carry the router inputs (x, residual,
        # router weights); bulk weights follow in compute order.  The
        # gpsimd queue is left free for the collectives + OUT copies.
        rw = wres.tile([128, KD * 2 * E], F16, tag="rw")
        nc.sync.dma_start(rw[:], rwP[:])
        aux_sb = wres.tile([128, E], F32, tag="aux")
        nc.sync.dma_start(aux_sb[:], AUX[:])
        alpha_bc = aux_sb[:, 0:E]

        xt = wres.tile([128, KD * CT], F16, tag="xt")
        nc.sync.dma_start(xt[:], xP[:])
        xr = wres.tile([128, KD * CT], F16, tag="xr")
        nc.scalar.dma_start(xr[:], xRP[:])

        gq = wres.tile([128, NE * KD * DF_E], F16, tag="gq")
        uq = wres.tile([128, NE * KD * DF_E], F16, tag="uq")
        dq = wres.tile([128, NE * KF * D], F16, tag="dq")
        wg = wres.tile([128, KD * FS], F16, tag="wg")
        wu_t = wres.tile([128, KD * FS], F16, tag="wu")
        wd = wres.tile([128, KS * D], F16, tag="wd")
        EW = KD * DF_E  # 4096 cols per expert in gq/uq
        DW = KF * D     # 4096 cols per expert in dq
        for e in range(NE):
            nc.sync.dma_start(gq[:, e * EW : (e + 1) * EW], GQ[:, e * EW : (e + 1) * EW])
            nc.scalar.dma_start(uq[:, e * EW : (e + 1) * EW], UQ[:, e * EW : (e + 1) * EW])
        nc.sync.dma_start(wg[:], WG[:])
        nc.scalar.dma_start(wu_t[:], WU[:])
        for e in range(NE):
            (nc.sync if e % 2 == 0 else nc.scalar).dma_start(
                dq[:, e * DW : (e + 1) * DW], DQ[:, e * DW : (e + 1) * DW]
            )
        nc.sync.dma_start(wd[:], WD[:])

        def xk(k):
            return xt[:, k * CT : (k + 1) * CT]

        def xrk(k):
            return xr[:, k * CT : (k + 1) * CT]

        # ---- router: split-precision fp16 logits ---------------------
        ps_lt = ps_r.tile([E, CT], F32, tag="psr_lt")
        for k in range(KD):
            nc.tensor.matmul(
                ps_lt[:], rw[:, k * 2 * E : k * 2 * E + E], xk(k),
                start=(k == 0), stop=False,
            )
        for k in range(KD):
            nc.tensor.matmul(
                ps_lt[:], rw[:, k * 2 * E + E : (k + 1) * 2 * E], xk(k),
                start=False, stop=False,
            )
        for k in range(KD):
            nc.tensor.matmul(
                ps_lt[:], rw[:, k * 2 * E : k * 2 * E + E], xrk(k),
                start=False, stop=(k == KD - 1),
            )

        # ---- combine weights (top-2 softmax * alpha) -----------------
        Lt = rt.tile([E, CT], F32, tag="Lt")
        nc.vector.tensor_copy(Lt[:], ps_lt[:])
        ps_l = ps_r.tile([128, TT * E], F32, tag="psr_l")
        for j in range(TT):
            nc.tensor.transpose(
                ps_l[:, j * E : (j + 1) * E],
                Lt[:, j * 128 : (j + 1) * 128],
                ident[0:E, 0:E],
            )
        L = rt.tile([128, TT * E], F32, tag="L")
        nc.vector.tensor_copy(L[:], ps_l[:])
        L3 = L[:].rearrange("p (j e) -> p j e", e=E)

        def bc(t):  # [128, TT] -> [128, TT, E] free-axis broadcast
            return t[:, :, None].broadcast_to([128, TT, E])

        m1 = rt.tile([128, TT], F32, tag="m1")
        nc.vector.tensor_reduce(m1[:], L3, mybir.AxisListType.X, ALU.max)
        mask1 = rt.tile([128, TT * E], F32, tag="mask1")
        mask1_3 = mask1[:].rearrange("p (j e) -> p j e", e=E)
        nc.vector.tensor_tensor(mask1_3, L3, bc(m1), op=ALU.is_ge)
        L2 = rt.tile([128, TT * E], F32, tag="L2")
        nc.vector.scalar_tensor_tensor(
            L2[:], mask1[:], -1e30, L[:], ALU.mult, ALU.add
        )
        L2_3 = L2[:].rearrange("p (j e) -> p j e", e=E)
        m2 = rt.tile([128, TT], F32, tag="m2")
        nc.vector.tensor_reduce(m2[:], L2_3, mybir.AxisListType.X, ALU.max)
        mask2 = rt.tile([128, TT * E], F32, tag="mask2")
        mask2_3 = mask2[:].rearrange("p (j e) -> p j e", e=E)
        nc.vector.tensor_tensor(mask2_3, L2_3, bc(m2), op=ALU.is_ge)
        # softmax over {m1, m2}: w1 = sigmoid(m1 - m2), w2 = 1 - w1
        dlt = rt.tile([128, TT], F32, tag="dlt")
        nc.vector.tensor_sub(dlt[:], m1[:], m2[:])
        w1 = rt.tile([128, TT], F32, tag="w1")
        nc.scalar.activation(w1[:], dlt[:], ACTF.Sigmoid)
        w2 = rt.tile([128, TT], F32, tag="w2")
        nc.vector.tensor_scalar(w2[:], w1[:], -1.0, 1.0, ALU.mult, ALU.add)
        caw = rt.tile([128, TT * E], F32, tag="caw")
        caw3 = caw[:].rearrange("p (j e) -> p j e", e=E)
        nc.vector.tensor_tensor(caw3, mask2_3, bc(w2), op=ALU.mult)
        t1 = rt.tile([128, TT * E], F32, tag="t1")
        t1_3 = t1[:].rearrange("p (j e) -> p j e", e=E)
        nc.vector.tensor_tensor(t1_3, mask1_3, bc(w1), op=ALU.mult)
        nc.vector.tensor_add(caw[:], caw[:], t1[:])
        # scale by alpha (broadcast over token-tiles) and reduce
        ca_a = rt.tile([128, TT * E], F32, tag="ca_a")
        ca_a3 = ca_a[:].rearrange("p (j e) -> p j e", e=E)
        alpha3 = alpha_bc[:, None, :].broadcast_to([128, TT, E])
        nc.vector.tensor_tensor(ca_a3, caw3, alpha3, op=ALU.mult)
        s = rt.tile([128, TT], F32, tag="s")
        nc.vector.tensor_reduce(s[:], ca_a3, mybir.AxisListType.X, ALU.add)
        om_all = rt.tile([128, TT], F32, tag="om")
        nc.vector.tensor_scalar(om_all[:], s[:], -1.0, 1.0, ALU.mult, ALU.add)

        # ---- gate/up + SwiGLU for 4 experts + shared half ------------
        def gu_block(wsrc_g, wsrc_u, base, f, tag):
            """one 128-row f-tile of gate/up + silu -> h tile (f16)."""
            psg = ps_gu.tile([128, CT], F32, tag="psgu")
            for k in range(KD):
                nc.tensor.matmul(
                    psg[:],
                    wsrc_g[:, base + k * stride + f * 128 : base + k * stride + f * 128 + 128],
                    xk(k),
                    start=(k == 0),
                    stop=(k == KD - 1),
                )
            psu = ps_gu.tile([128, CT], F32, tag="psgu")
            for k in range(KD):
                nc.tensor.matmul(
                    psu[:],
                    wsrc_u[:, base + k * stride + f * 128 : base + k * stride + f * 128 + 128],
                    xk(k),
                    start=(k == 0),
                    stop=(k == KD - 1),
                )
            sig = work.tile([128, CT], F32, tag="sig")
            nc.scalar.activation(sig[:], psg[:], ACTF.Sigmoid)
            sil = work.tile([128, CT], F32, tag="sil")
            nc.vector.tensor_mul(sil[:], sig[:], psg[:])
            h = hp.tile([128, CT], F16, tag=tag)
            nc.vector.tensor_mul(h[:], sil[:], psu[:])
            return h

        he = {}
        stride = DF_E  # within one expert block, k-tiles are DF_E wide
        for e in range(NE):
            for f in range(FE):
                he[(e, f)] = gu_block(gq, uq, e * EW, f, f"he{e}_{f}")
        hs = {}
        stride = FS
        for f in range(KS):
            hs[f] = gu_block(wg, wu_t, 0, f, f"hs{f}")

        # ---- down projections + combine, 2 column halves -------------
        ND = D // 512
        rs_in = [dram.tile([CT, 512], F16, tag=f"rsin{dd}", name=f"rsin{dd}") for dd in range(ND)]
        rs_out = [dram.tile([CT // 2, 512], F16, tag=f"rsout{dd}", name=f"rsout{dd}") for dd in range(ND)]
        for dd in range(ND):
            for j in range(TT):
                pse = []
                for e in range(NE):
                    p = ps_dn.tile([128, 512], F32, tag="psd")
                    for k in range(KF):
                        nc.tensor.matmul(
                            p[:],
                            he[(e, k)][:, j * 128 : (j + 1) * 128],
                            dq[:, e * DW + k * D + dd * 512 : e * DW + k * D + dd * 512 + 512],
                            start=(k == 0),
                            stop=(k == KF - 1),
                        )
                    pse.append(p)
                pss = ps_dn.tile([128, 512], F32, tag="psd")
                for k in range(KS):
                    nc.tensor.matmul(
                        pss[:],
                        hs[k][:, j * 128 : (j + 1) * 128],
                        wd[:, k * D + dd * 512 : k * D + dd * 512 + 512],
                        start=(k == 0),
                        stop=(k == KS - 1),
                    )
                # contrib = sum_e ca_e*pse_e + (1-s)*pss  (ca cols 0..3)
                def cacol(e):
                    return ca_a[:, j * E + e : j * E + e + 1]

                acc_a = work.tile([128, 512], F32, tag="acc_a")
                nc.vector.tensor_scalar(acc_a[:], pse[0][:], cacol(0), None, ALU.mult)
                acc_b = work.tile([128, 512], F32, tag="acc_b")
                nc.vector.scalar_tensor_tensor(
                    acc_b[:], pse[1][:], cacol(1), acc_a[:], ALU.mult, ALU.add
                )
                acc_c = work.tile([128, 512], F32, tag="acc_a")
                nc.vector.scalar_tensor_tensor(
                    acc_c[:], pse[2][:], cacol(2), acc_b[:], ALU.mult, ALU.add
                )
                acc_d = work.tile([128, 512], F32, tag="acc_b")
                nc.vector.scalar_tensor_tensor(
                    acc_d[:], pse[3][:], cacol(3), acc_c[:], ALU.mult, ALU.add
                )
                contrib = work.tile([128, 512], F16, tag="contrib")
                nc.vector.scalar_tensor_tensor(
                    contrib[:],
                    pss[:],
                    om_all[:, j : j + 1],
                    acc_d[:],
                    ALU.mult,
                    ALU.add,
                )
                nc.scalar.dma_start(rs_in[dd][j * 128 : (j + 1) * 128, :], contrib[:])

            nc.gpsimd.collective_compute(
                "ReduceScatter",
                ALU.add,
                replica_groups=PAIR_GROUPS,
                ins=[rs_in[dd].opt()],
                outs=[rs_out[dd].opt()],
            )
            nc.gpsimd.dma_start(OUT[:, dd * 512 : (dd + 1) * 512], rs_out[dd][:])

    nc.compile()
    return nc


def _pack(a):
    """[k*128, C] row-major -> [128, k*C] tile-packed layout."""
    R, C = a.shape
    k = R // 128
    return np.ascontiguousarray(
        a.reshape(k, 128, C).transpose(1, 0, 2).reshape(128, k * C)
    )


def _prep_inputs(x, router_weight, sh_gate_w, sh_up_w, sh_down_w, gate_s,
                 up_s, down_s, alpha, gate_q, up_q, down_q):
    xf32 = np.asarray(x, dtype=np.float32).reshape(T, D).T  # [D, T]
    xf = xf32.astype(np.float16)
    xres = (xf32 - xf.astype(np.float32)).astype(np.float16)

    rw32 = np.asarray(router_weight, np.float32).T  # [D, E]

    # parity-dependent data: expert permutation + shared half
    par = {}
    for p in range(2):
        perm = list(range(E)) if p == 0 else list(range(4, 8)) + list(range(4))
        rw_p = rw32[:, perm]
        rw_hi = rw_p.astype(np.float16)
        rw_lo = (rw_p - rw_hi.astype(np.float32)).astype(np.float16)
        # interleave per k-tile: [D, 2E] with hi|lo per k after packing
        rwPk = np.concatenate([rw_hi, rw_lo], axis=1)  # [D, 16]
        gq_l, uq_l, dq_l = [], [], []
        for e in perm[:NE]:
            gw = (np.asarray(gate_q[e], np.float32)
                  * np.asarray(gate_s[e], np.float32)[:, None])  # [DF_E, D]
            uw = (np.asarray(up_q[e], np.float32)
                  * np.asarray(up_s[e], np.float32)[:, None])
            dw = (np.asarray(down_q[e], np.float32)
                  * np.asarray(down_s[e], np.float32)[:, None])  # [D, DF_E]
            gq_l.append(_pack(gw.T.astype(np.float16)))   # [128, KD*DF_E]
            uq_l.append(_pack(uw.T.astype(np.float16)))
            dq_l.append(_pack(dw.T.astype(np.float16)))   # [128, KF*D]
        aux = np.zeros((128, E), np.float32)
        aux[:, :] = np.asarray(alpha, np.float32)[perm][None, :]
        par[p] = {
            "rwP": _pack(rwPk.astype(np.float16)),
            "GQ": np.ascontiguousarray(np.concatenate(gq_l, axis=1)),
            "UQ": np.ascontiguousarray(np.concatenate(uq_l, axis=1)),
            "DQ": np.ascontiguousarray(np.concatenate(dq_l, axis=1)),
            "WG": _pack(
                np.asarray(sh_gate_w[p * FS : (p + 1) * FS], np.float32)
                .T.astype(np.float16)
            ),
            "WU": _pack(
                np.asarray(sh_up_w[p * FS : (p + 1) * FS], np.float32)
                .T.astype(np.float16)
            ),
            "WD": _pack(
                np.asarray(sh_down_w[:, p * FS : (p + 1) * FS], np.float32)
                .T.astype(np.float16)
            ),
            "AUX": aux,
        }
    xs = {}
    for m in range(NPAIR):
        xs[m] = {
            "xP": _pack(xf[:, m * CT : (m + 1) * CT]),
            "xRP": _pack(xres[:, m * CT : (m + 1) * CT]),
        }
    in_maps = []
    for c in range(NCORES):
        d = dict(par[c % 2])
        d.update(xs[c // 2])
        in_maps.append(d)
    return in_maps


def assemble(outs):
    """Per-core OUT [256, D]: core 2m+p holds tokens [512m+256p, +256)."""
    out = np.empty((T, D), np.float32)
    half = CT // 2
    for c in range(NCORES):
        m, p = c // 2, c % 2
        base = m * CT + p * half
        out[base : base + half] = np.asarray(outs[c])
    return out.reshape(B, S, D)


def kernel(x, router_weight, sh_gate_w, sh_up_w, sh_down_w, gate_s, up_s,
           down_s, alpha, gate_q, up_q, down_q, top_k, **run_kwargs):
    assert int(top_k) == 2, "kernel compiled for top_k=2"
    assert tuple(np.shape(x)) == (B, S, D)

    if "nc" not in _CACHE:
        _CACHE["nc"] = _build()
    nc = _CACHE["nc"]

    in_maps = _prep_inputs(
        x, router_weight, sh_gate_w, sh_up_w, sh_down_w, gate_s, up_s,
        down_s, alpha, gate_q, up_q, down_q,
    )
    res = run_bass_kernel_spmd(
        nc, in_maps, core_ids=list(range(NCORES)), **run_kwargs
    )
    _CACHE["last_results"] = res

    outs = [res.results[r]["OUT"] for r in range(NCORES)]
    return assemble(outs).astype(np.asarray(x).dtype)


# revision 4
# speedup vs baseline: 1.1772x; 1.0695x over previous
"""Routed quantized MoE eval kernel for 8 Trainium2 NeuronCores.

Hybrid pair-parallel strategy (v2, replaces 8-way expert-parallel):
- Cores are grouped in 4 pairs; pair m owns tokens [512m, 512m+512).
  Within a pair, the even core computes experts {0..3} and shared-FFN
  rows [0:1024), the odd core experts {4..7} and rows [1024:2048).
  The pair-sum is taken by a 2-rank ReduceScatter per 512-column half
  of the output (first RS hidden under the second half's compute).
- Expert order is PERMUTED per parity (odd cores see experts
  [4,5,6,7,0,1,2,3] in router weights, alpha and weight uploads), so
  the SPMD program always works on combine columns 0..3.  Top-2
  selection and softmax are permutation-invariant.
- All matmuls keep 512-wide moving operands (PE instruction overhead
  ~50ns each - fewer, wider matmuls win).  Router uses the same
  split-precision fp16 trick as before (3 exact fp16 products in one
  fp32 PSUM).
- Per-(token-tile, dd) combine: contrib = sum_e ca_e*alpha_e*pse_e
  + (1-s)*pss via a chain of scalar_tensor_tensor ops on the DVE.
- Weights are host-dequantized (scale-folded), transposed, and packed
  to a [128, k*C] tile layout so every tensor is one large DMA.
"""

import numpy as np
from contextlib import ExitStack

import concourse.bass as bass
import concourse.tile as tile
from concourse import bacc, mybir
from concourse.bass_utils import run_bass_kernel_spmd

NCORES = 8
B, S, D = 2, 1024, 1024
T = B * S                      # 2048 tokens
DF_E, DF_S, E = 512, 2048, 8
NPAIR = 4
CT = T // NPAIR                # 512 tokens per pair
TT = CT // 128                 # 4 token tiles
NE = 4                         # experts per core
FS = DF_S // 2                 # 1024 shared-ffn rows per core
KD = D // 128                  # 8 k-tiles over hidden dim
KF = DF_E // 128               # 4 k-tiles over expert ffn dim
KS = FS // 128                 # 8 k-tiles over shared ffn shard
FE = DF_E // 128               # 4 f-tiles per expert

F16 = mybir.dt.float16
F32 = mybir.dt.float32
ACTF = mybir.ActivationFunctionType
ALU = mybir.AluOpType

PAIR_GROUPS = [[2 * m, 2 * m + 1] for m in range(NPAIR)]

_CACHE = {}


def _build():
    nc = bacc.Bacc(
        "TRN2", target_bir_lowering=False, debug=False, num_devices=NCORES
    )

    # all weight tensors are packed [128, k*C]: column block k holds rows
    # [128k, 128k+128) of the logical [k*128, C] matrix
    xP = nc.dram_tensor("xP", [128, KD * CT], F16, kind="ExternalInput").ap()
    xRP = nc.dram_tensor("xRP", [128, KD * CT], F16, kind="ExternalInput").ap()
    rwP = nc.dram_tensor("rwP", [128, KD * 2 * E], F16, kind="ExternalInput").ap()
    GQ = nc.dram_tensor("GQ", [128, NE * KD * DF_E], F16, kind="ExternalInput").ap()
    UQ = nc.dram_tensor("UQ", [128, NE * KD * DF_E], F16, kind="ExternalInput").ap()
    DQ = nc.dram_tensor("DQ", [128, NE * KF * D], F16, kind="ExternalInput").ap()
    WG = nc.dram_tensor("WG", [128, KD * FS], F16, kind="ExternalInput").ap()
    WU = nc.dram_tensor("WU", [128, KD * FS], F16, kind="ExternalInput").ap()
    WD = nc.dram_tensor("WD", [128, KS * D], F16, kind="ExternalInput").ap()
    AUX = nc.dram_tensor("AUX", [128, E], F32, kind="ExternalInput").ap()
    OUT = nc.dram_tensor("OUT", [CT // 2, D], F16, kind="ExternalOutput").ap()

    with ExitStack() as ctx:
        tc = ctx.enter_context(tile.TileContext(nc))
        wres = ctx.enter_context(tc.tile_pool(name="wres", bufs=1))
        hp = ctx.enter_context(tc.tile_pool(name="hp", bufs=1))
        work = ctx.enter_context(tc.tile_pool(name="work", bufs=2))
        rt = ctx.enter_context(tc.tile_pool(name="rt", bufs=1))
        ps_gu = ctx.enter_context(tc.tile_pool(name="ps_gu", bufs=3, space="PSUM"))
        ps_dn = ctx.enter_context(tc.tile_pool(name="ps_dn", bufs=3, space="PSUM"))
        ps_r = ctx.enter_context(tc.tile_pool(name="ps_r", bufs=1, space="PSUM"))
        dram = ctx.enter_context(tc.tile_pool(name="dram", bufs=1, space="DRAM"))

        from concourse.masks import make_identity

        # warmup ReduceScatter FIRST on the in-order gpsimd queue: the
        # ncfw cold-start begins at trigger time, so trigger ASAP
        wu_in = dram.tile([2, 128], F16, tag="wuin")
        wu_out = dram.tile([1, 128], F16, tag="wuout")
        nc.sync.dma_start(wu_in[:], xP[0:2, 0:128])
        nc.gpsimd.collective_compute(
            "ReduceScatter",
            ALU.add,
            replica_groups=PAIR_GROUPS,
            ins=[wu_in.opt()],
            outs=[wu_out.opt()],
        )
        ident = wres.tile([128, 128], F32, tag="ident")
        make_identity(nc, ident[:])
        # dummy matmuls during the initial DMA wait: warms the PE clock.
        # Reuses a ps_gu-pool generation to stay within the 8 PSUM banks.
        wu_ps = ps_gu.tile([128, CT], F32, tag="psgu")
        for _ in range(16):
            nc.tensor.matmul(
                wu_ps[:, 0:128], ident[:], ident[:], start=True, stop=True
            )

        # ---- input DMAs: two queues, router-critical first ------------
        # DMA transfers serialize globally in arrival order, so the
        # sync/scalar queue heads carry the router inputs (x, residual,
        # router weights); bulk weights follow in compute order.  The
        # gpsimd queue is left free for the collectives + OUT copies.
        rw = wres.tile([128, KD * 2 * E], F16, tag="rw")
        nc.sync.dma_start(rw[:], rwP[:])
        aux_sb = wres.tile([128, E], F32, tag="aux")
        nc.sync.dma_start(aux_sb[:], AUX[:])
        alpha_bc = aux_sb[:, 0:E]

        xt = wres.tile([128, KD * CT], F16, tag="xt")
        nc.sync.dma_start(xt[:], xP[:])
        xr = wres.tile([128, KD * CT], F16, tag="xr")
        nc.scalar.dma_start(xr[:], xRP[:])

        gq = wres.tile([128, NE * KD * DF_E], F16, tag="gq")
        uq = wres.tile([128, NE * KD * DF_E], F16, tag="uq")
        dq = wres.tile([128, NE * KF * D], F16, tag="dq")
        wg = wres.tile([128, KD * FS], F16, tag="wg")
        wu_t = wres.tile([128, KD * FS], F16, tag="wu")
        wd = wres.tile([128, KS * D], F16, tag="wd")
        EW = KD * DF_E  # 4096 cols per expert in gq/uq
        DW = KF * D     # 4096 cols per expert in dq
        for e in range(NE):
            nc.sync.dma_start(gq[:, e * EW : (e + 1) * EW], GQ[:, e * EW : (e + 1) * EW])
            nc.scalar.dma_start(uq[:, e * EW : (e + 1) * EW], UQ[:, e * EW : (e + 1) * EW])
        nc.sync.dma_start(wg[:], WG[:])
        nc.scalar.dma_start(wu_t[:], WU[:])
        for e in range(NE):
            (nc.sync if e % 2 == 0 else nc.scalar).dma_start(
                dq[:, e * DW : (e + 1) * DW], DQ[:, e * DW : (e + 1) * DW]
            )
        nc.sync.dma_start(wd[:], WD[:])

        def xk(k):
            return xt[:, k * CT : (k + 1) * CT]

        def xrk(k):
            return xr[:, k * CT : (k + 1) * CT]

        # ---- router: split-precision fp16 logits ---------------------
        ps_lt = ps_r.tile([E, CT], F32, tag="psr_lt")
        for k in range(KD):
            nc.tensor.matmul(
                ps_lt[:], rw[:, k * 2 * E : k * 2 * E + E], xk(k),
                start=(k == 0), stop=False,
            )
        for k in range(KD):
            nc.tensor.matmul(
                ps_lt[:], rw[:, k * 2 * E + E : (k + 1) * 2 * E], xk(k),
                start=False, stop=False,
            )
        for k in range(KD):
            nc.tensor.matmul(
                ps_lt[:], rw[:, k * 2 * E : k * 2 * E + E], xrk(k),
                start=False, stop=(k == KD - 1),
            )

        # ---- combine weights (top-2 softmax * alpha) -----------------
        Lt = rt.tile([E, CT], F32, tag="Lt")
        nc.vector.tensor_copy(Lt[:], ps_lt[:])
        ps_l = ps_r.tile([128, TT * E], F32, tag="psr_l")
        for j in range(TT):
            nc.tensor.transpose(
                ps_l[:, j * E : (j + 1) * E],
                Lt[:, j * 128 : (j + 1) * 128],
                ident[0:E, 0:E],
            )
        L = rt.tile([128, TT * E], F32, tag="L")
        nc.vector.tensor_copy(L[:], ps_l[:])
        L3 = L[:].rearrange("p (j e) -> p j e", e=E)

        def bc(t):  # [128, TT] -> [128, TT, E] free-axis broadcast
            return t[:, :, None].broadcast_to([128, TT, E])

        m1 = rt.tile([128, TT], F32, tag="m1")
        nc.vector.tensor_reduce(m1[:], L3, mybir.AxisListType.X, ALU.max)
        mask1 = rt.tile([128, TT * E], F32, tag="mask1")
        mask1_3 = mask1[:].rearrange("p (j e) -> p j e", e=E)
        nc.vector.tensor_tensor(mask1_3, L3, bc(m1), op=ALU.is_ge)
        L2 = rt.tile([128, TT * E], F32, tag="L2")
        nc.vector.scalar_tensor_tensor(
            L2[:], mask1[:], -1e30, L[:], ALU.mult, ALU.add
        )
        L2_3 = L2[:].rearrange("p (j e) -> p j e", e=E)
        m2 = rt.tile([128, TT], F32, tag="m2")
        nc.vector.tensor_reduce(m2[:], L2_3, mybir.AxisListType.X, ALU.max)
        mask2 = rt.tile([128, TT * E], F32, tag="mask2")
        mask2_3 = mask2[:].rearrange("p (j e) -> p j e", e=E)
        nc.vector.tensor_tensor(mask2_3, L2_3, bc(m2), op=ALU.is_ge)
        # softmax over {m1, m2}: w1 = sigmoid(m1 - m2), w2 = 1 - w1
        dlt = rt.tile([128, TT], F32, tag="dlt")
        nc.vector.tensor_sub(dlt[:], m1[:], m2[:])
        w1 = rt.tile([128, TT], F32, tag="w1")
        nc.scalar.activation(w1[:], dlt[:], ACTF.Sigmoid)
        w2 = rt.tile([128, TT], F32, tag="w2")
        nc.vector.tensor_scalar(w2[:], w1[:], -1.0, 1.0, ALU.mult, ALU.add)
        caw = rt.tile([128, TT * E], F32, tag="caw")
        caw3 = caw[:].rearrange("p (j e) -> p j e", e=E)
        nc.vector.tensor_tensor(caw3, mask2_3, bc(w2), op=ALU.mult)
        t1 = rt.tile([128, TT * E], F32, tag="t1")
        t1_3 = t1[:].rearrange("p (j e) -> p j e", e=E)
        nc.vector.tensor_tensor(t1_3, mask1_3, bc(w1), op=ALU.mult)
        nc.vector.tensor_add(caw[:], caw[:], t1[:])
        # scale by alpha (broadcast over token-tiles) and reduce
        ca_a = rt.tile([128, TT * E], F32, tag="ca_a")
        ca_a3 = ca_a[:].rearrange("p (j e) -> p j e", e=E)
        alpha3 = alpha_bc[:, None, :].broadcast_to([128, TT, E])
        nc.vector.tensor_tensor(ca_a3, caw3, alpha3, op=ALU.mult)
        s = rt.tile([128, TT], F32, tag="s")
        nc.vector.tensor_reduce(s[:], ca_a3, mybir.AxisListType.X, ALU.add)
        om_all = rt.tile([128, TT], F32, tag="om")
        nc.vector.tensor_scalar(om_all[:], s[:], -1.0, 1.0, ALU.mult, ALU.add)

        # ---- gate/up + SwiGLU for 4 experts + shared half ------------
        def gu_block(wsrc_g, wsrc_u, base, f, tag):
            """one 128-row f-tile of gate/up + silu -> h tile (f16)."""
            psg = ps_gu.tile([128, CT], F32, tag="psgu")
            for k in range(KD):
                nc.tensor.matmul(
                    psg[:],
                    wsrc_g[:, base + k * stride + f * 128 : base + k * stride + f * 128 + 128],
                    xk(k),
                    start=(k == 0),
                    stop=(k == KD - 1),
                )
            psu = ps_gu.tile([128, CT], F32, tag="psgu")
            for k in range(KD):
                nc.tensor.matmul(
                    psu[:],
                    wsrc_u[:, base + k * stride + f * 128 : base + k * stride + f * 128 + 128],
                    xk(k),
                    start=(k == 0),
                    stop=(k == KD - 1),
                )
            sig = work.tile([128, CT], F32, tag="sig")
            nc.scalar.activation(sig[:], psg[:], ACTF.Sigmoid)
            sil = work.tile([128, CT], F32, tag="sil")
            nc.vector.tensor_mul(sil[:], sig[:], psg[:])
            h = hp.tile([128, CT], F16, tag=tag)
            nc.vector.tensor_mul(h[:], sil[:], psu[:])
            return h

        he = {}
        stride = DF_E  # within one expert block, k-tiles are DF_E wide
        for e in range(NE):
            for f in range(FE):
                he[(e, f)] = gu_block(gq, uq, e * EW, f, f"he{e}_{f}")
        hs = {}
        stride = FS
        for f in range(KS):
            hs[f] = gu_block(wg, wu_t, 0, f, f"hs{f}")

        # ---- down projections + combine, 2 column halves -------------
        ND = D // 512
        rs_in = [dram.tile([CT, 512], F16, tag=f"rsin{dd}", name=f"rsin{dd}") for dd in range(ND)]
        rs_out = [dram.tile([CT // 2, 512], F16, tag=f"rsout{dd}", name=f"rsout{dd}") for dd in range(ND)]
        for dd in range(ND):
            for j in range(TT):
                pse = []
                for e in range(NE):
                    p = ps_dn.tile([128, 512], F32, tag="psd")
                    for k in range(KF):
                        nc.tensor.matmul(
                            p[:],
                            he[(e, k)][:, j * 128 : (j + 1) * 128],
                            dq[:, e * DW + k * D + dd * 512 : e * DW + k * D + dd * 512 + 512],
                            start=(k == 0),
                            stop=(k == KF - 1),
                        )
                    pse.append(p)
                pss = ps_dn.tile([128, 512], F32, tag="psd")
                for k in range(KS):
                    nc.tensor.matmul(
                        pss[:],
                        hs[k][:, j * 128 : (j + 1) * 128],
                        wd[:, k * D + dd * 512 : k * D + dd * 512 + 512],
                        start=(k == 0),
                        stop=(k == KS - 1),
                    )
                # contrib = sum_e ca_e*pse_e + (1-s)*pss  (ca cols 0..3)
                def cacol(e):
                    return ca_a[:, j * E + e : j * E + e + 1]

                acc_a = work.tile([128, 512], F32, tag="acc_a")
                nc.vector.tensor_scalar(acc_a[:], pse[0][:], cacol(0), None, ALU.mult)
                acc_b = work.tile([128, 512], F32, tag="acc_b")
                nc.vector.scalar_tensor_tensor(
                    acc_b[:], pse[1][:], cacol(1), acc_a[:], ALU.mult, ALU.add
                )
                acc_c = work.tile([128, 512], F32, tag="acc_a")
                nc.vector.scalar_tensor_tensor(
                    acc_c[:], pse[2][:], cacol(2), acc_b[:], ALU.mult, ALU.add
                )
                acc_d = work.tile([128, 512], F32, tag="acc_b")
                nc.vector.scalar_tensor_tensor(
                    acc_d[:], pse[3][:], cacol(3), acc_c[:], ALU.mult, ALU.add
                )
                contrib = work.tile([128, 512], F16, tag="contrib")
                nc.vector.scalar_tensor_tensor(
                    contrib[:],
                    pss[:],
                    om_all[:, j : j + 1],
                    acc_d[:],
                    ALU.mult,
                    ALU.add,
                )
                nc.scalar.dma_start(rs_in[dd][j * 128 : (j + 1) * 128, :], contrib[:])

            nc.gpsimd.collective_compute(
                "ReduceScatter",
                ALU.add,
                replica_groups=PAIR_GROUPS,
                ins=[rs_in[dd].opt()],
                outs=[rs_out[dd].opt()],
            )
            nc.gpsimd.dma_start(OUT[:, dd * 512 : (dd + 1) * 512], rs_out[dd][:])

    nc.compile()
    return nc


def _pack(a):
    """[k*128, C] row-major -> [128, k*C] tile-packed layout."""
    R, C = a.shape
    k = R // 128
    return np.ascontiguousarray(
        a.reshape(k, 128, C).transpose(1, 0, 2).reshape(128, k * C)
    )


def _prep_inputs(x, router_weight, sh_gate_w, sh_up_w, sh_down_w, gate_s,
                 up_s, down_s, alpha, gate_q, up_q, down_q):
    xf32 = np.asarray(x, dtype=np.float32).reshape(T, D).T  # [D, T]
    xf = xf32.astype(np.float16)
    xres = (xf32 - xf.astype(np.float32)).astype(np.float16)

    rw32 = np.asarray(router_weight, np.float32).T  # [D, E]

    # parity-dependent data: expert permutation + shared half
    par = {}
    for p in range(2):
        perm = list(range(E)) if p == 0 else list(range(4, 8)) + list(range(4))
        rw_p = rw32[:, perm]
        rw_hi = rw_p.astype(np.float16)
        rw_lo = (rw_p - rw_hi.astype(np.float32)).astype(np.float16)
        # interleave per k-tile: [D, 2E] with hi|lo per k after packing
        rwPk = np.concatenate([rw_hi, rw_lo], axis=1)  # [D, 16]
        gq_l, uq_l, dq_l = [], [], []
        for e in perm[:NE]:
            gw = (np.asarray(gate_q[e], np.float32)
                  * np.asarray(gate_s[e], np.float32)[:, None])  # [DF_E, D]
            uw = (np.asarray(up_q[e], np.float32)
                  * np.asarray(up_s[e], np.float32)[:, None])
            dw = (np.asarray(down_q[e], np.float32)
                  * np.asarray(down_s[e], np.float32)[:, None])  # [D, DF_E]
            gq_l.append(_pack(gw.T.astype(np.float16)))   # [128, KD*DF_E]
            uq_l.append(_pack(uw.T.astype(np.float16)))
            dq_l.append(_pack(dw.T.astype(np.float16)))   # [128, KF*D]
        aux = np.zeros((128, E), np.float32)
        aux[:, :] = np.asarray(alpha, np.float32)[perm][None, :]
        par[p] = {
            "rwP": _pack(rwPk.astype(np.float16)),
            "GQ": np.ascontiguousarray(np.concatenate(gq_l, axis=1)),
            "UQ": np.ascontiguousarray(np.concatenate(uq_l, axis=1)),
            "DQ": np.ascontiguousarray(np.concatenate(dq_l, axis=1)),
            "WG": _pack(
                np.asarray(sh_gate_w[p * FS : (p + 1) * FS], np.float32)
                .T.astype(np.float16)
            ),
            "WU": _pack(
                np.asarray(sh_up_w[p * FS : (p + 1) * FS], np.float32)
                .T.astype(np.float16)
            ),
            "WD": _pack(
                np.asarray(sh_down_w[:, p * FS : (p + 1) * FS], np.float32)
                .T.astype(np.float16)
            ),
            "AUX": aux,
        }
    xs = {}
    for m in range(NPAIR):
        xs[m] = {
            "xP": _pack(xf[:, m * CT : (m + 1) * CT]),
            "xRP": _pack(xres[:, m * CT : (m + 1) * CT]),
        }
    in_maps = []
    for c in range(NCORES):
        d = dict(par[c % 2])
        d.update(xs[c // 2])
        in_maps.append(d)
    return in_maps


def assemble(outs):
    """Per-core OUT [256, D]: core 2m+p holds tokens [512m+256p, +256)."""
    out = np.empty((T, D), np.float32)
    half = CT // 2
    for c in range(NCORES):
        m, p = c // 2, c % 2
        base = m * CT + p * half
        out[base : base + half] = np.asarray(outs[c])
    return out.reshape(B, S, D)


def kernel(x, router_weight, sh_gate_w, sh_up_w, sh_down_w, gate_s, up_s,
           down_s, alpha, gate_q, up_q, down_q, top_k, **run_kwargs):
    assert int(top_k) == 2, "kernel compiled for top_k=2"
    assert tuple(np.shape(x)) == (B, S, D)

    if "nc" not in _CACHE:
        _CACHE["nc"] = _build()
    nc = _CACHE["nc"]

    in_maps = _prep_inputs(
        x, router_weight, sh_gate_w, sh_up_w, sh_down_w, gate_s, up_s,
        down_s, alpha, gate_q, up_q, down_q,
    )
    res = run_bass_kernel_spmd(
        nc, in_maps, core_ids=list(range(NCORES)), **run_kwargs
    )
    _CACHE["last_results"] = res

    outs = [res.results[r]["OUT"] for r in range(NCORES)]
    return assemble(outs).astype(np.asarray(x).dtype)
